# revision 12
# baseline (speedup 1.0000x reference)
"""MultiHeadAttention Bass/Tile kernel for Trainium2, 8 NeuronCores SPMD.

Problem (hardcoded): B=2, S=2048, D=1024, H=16, d_k=64, fp32.
reference:
    qh = split_heads(q @ Wq.T + bq)  ... etc
    scores = qh @ kh.T / sqrt(d_k)
    attn = softmax(scores)
    out_h = attn @ vh
    output = merge_heads(out_h) @ Wo.T + bo
    returns (output, attn)

Sharding: core c handles batch b=c//4, heads 4*(c%4) .. 4*(c%4)+3
(data parallel over batch x tensor parallel over heads).
W_{q,k,v} split column-wise (256 output dims per core), Wo row-wise;
partial outputs summed on host (cheap: 4 x 8MiB adds per batch).

Per-core pipeline (all matmuls in float32r = hw-rounded fp32, ~1e-4 rel):
  - projections produce qh.T, kh.T ([n,s] layout) and vh ([t,n] layout)
  - per head: scores[s,t] tiles -> exp(scale=1/8) with accum row sums ->
    reciprocal -> normalize -> DMA attn out (natural layout)
  - per head: scores.T[t,s] tiles -> exp -> AV matmul accumulates
    out.T[d,s] (unnormalized); normalized afterwards with a broadcast
    recip row built via PE transpose + ones-outer-product
  - output projection from out.T with Wo.T, partial over this core's heads
"""

import numpy as np

import concourse.bass as bass
import concourse.mybir as mybir
import concourse.tile as tile
from concourse import bacc
from concourse.masks import make_identity
from concourse.bass_utils import run_bass_kernel_spmd

F32 = mybir.dt.float32
F32R = mybir.dt.float32r
AF = mybir.ActivationFunctionType

B, S, D, H, DK = 2, 2048, 1024, 16, 64
NCORES = 8
HPC = 4          # heads per core
NS = HPC * DK    # 256 hidden dims per core
KC = D // 128    # 8 contraction chunks for projections
SB = S // 128    # 16 seq partition blocks
SCALE = 1.0 / np.sqrt(np.float32(DK))

_CACHE = {}


def _build():
    nc = bacc.Bacc()

    xqT = nc.dram_tensor("xqT", [D, S], F32, kind="ExternalInput")
    xkT = nc.dram_tensor("xkT", [D, S], F32, kind="ExternalInput")
    xvT = nc.dram_tensor("xvT", [D, S], F32, kind="ExternalInput")
    wqT = nc.dram_tensor("wqT", [D, NS], F32, kind="ExternalInput")
    wkT = nc.dram_tensor("wkT", [D, NS], F32, kind="ExternalInput")
    wvT = nc.dram_tensor("wvT", [D, NS], F32, kind="ExternalInput")
    woT = nc.dram_tensor("woT", [NS, D], F32, kind="ExternalInput")
    bq_d = nc.dram_tensor("bq", [NS], F32, kind="ExternalInput")
    bk_d = nc.dram_tensor("bk", [NS], F32, kind="ExternalInput")
    bv_d = nc.dram_tensor("bv", [NS], F32, kind="ExternalInput")

    attn_d = nc.dram_tensor("attn_sh", [HPC, S, S], F32, kind="ExternalOutput")
    out_d = nc.dram_tensor("out_part", [S, D], F32, kind="ExternalOutput")

    with tile.TileContext(nc) as tc:
        from contextlib import ExitStack
        with ExitStack() as ctx:
            consts = ctx.enter_context(tc.tile_pool(name="consts", bufs=1))
            wpool = ctx.enter_context(tc.tile_pool(name="wpool", bufs=1))
            xt = ctx.enter_context(tc.tile_pool(name="xt", bufs=2))
            proj_sb = ctx.enter_context(tc.tile_pool(name="proj_sb", bufs=1))
            natp = ctx.enter_context(tc.tile_pool(name="natp", bufs=2))
            attp = ctx.enter_context(tc.tile_pool(name="attp", bufs=2))
            expTp = ctx.enter_context(tc.tile_pool(name="expTp", bufs=3))
            smalls = ctx.enter_context(tc.tile_pool(name="smalls", bufs=2))
            avp = ctx.enter_context(tc.tile_pool(name="avp", bufs=1))
            outp = ctx.enter_context(tc.tile_pool(name="outp", bufs=2))

            # ---------------- constants ----------------
            ident = consts.tile([128, 128], F32)
            make_identity(nc, ident[:])
            ones64 = consts.tile([128, 64], F32)
            nc.vector.memset(ones64[:], 1.0)
            bq_sb = consts.tile([128, 2], F32)
            bk_sb = consts.tile([128, 2], F32)
            nc.sync.dma_start(out=bq_sb[:], in_=bq_d.ap().rearrange("(m p) -> p m", p=128))
            nc.sync.dma_start(out=bk_sb[:], in_=bk_d.ap().rearrange("(m p) -> p m", p=128))
            bv_bc = consts.tile([128, NS], F32)
            bv_src = bass.AP(tensor=bv_d, offset=0, ap=[[0, 128], [1, NS]])
            nc.gpsimd.dma_start(out=bv_bc[:], in_=bv_src)

            # ---------------- weights (cast to f32r) ----------------
            wq_sb = wpool.tile([128, KC, NS], F32R)
            wk_sb = wpool.tile([128, KC, NS], F32R)
            wv_sb = wpool.tile([128, KC, NS], F32R)
            nc.gpsimd.dma_start(out=wq_sb[:], in_=wqT.ap().rearrange("(kc p) n -> p kc n", p=128))
            nc.gpsimd.dma_start(out=wk_sb[:], in_=wkT.ap().rearrange("(kc p) n -> p kc n", p=128))
            nc.gpsimd.dma_start(out=wv_sb[:], in_=wvT.ap().rearrange("(kc p) n -> p kc n", p=128))
            wo_sb = wpool.tile([128, 2, D], F32R)
            nc.gpsimd.dma_start(out=wo_sb[:], in_=woT.ap().rearrange("(nch p) o -> p nch o", p=128))

            # ---------------- projections ----------------
            qhT, khT = [], []   # 2 x [128, S] f32r each ([n, s])
            vh = []             # SB x [128, NS] f32r ([t, n])

            with tc.tile_pool(name="proj_ps", bufs=1, space="PSUM") as pps:
                # qh.T and kh.T : psum[m][n128, s] = sum_kc W*T[kc][:, m].T @ x*T[kc]
                for which, w_sb, x_d, b_sb, dst in (
                    ("q", wq_sb, xqT, bq_sb, qhT),
                    ("k", wk_sb, xkT, bk_sb, khT),
                ):
                    ps = [pps.tile([128, S], F32, name=f"ps_{which}{m}", tag=f"pp{m}")
                          for m in range(2)]
                    for kc in range(KC):
                        x_t = xt.tile([128, S], F32R, name=f"x_{which}{kc}", tag="xt")
                        nc.gpsimd.dma_start(out=x_t[:], in_=x_d.ap()[kc * 128:(kc + 1) * 128, :])
                        for m in range(2):
                            for nch in range(4):
                                nc.tensor.matmul(
                                    ps[m][:, nch * 512:(nch + 1) * 512],
                                    w_sb[:, kc, m * 128:(m + 1) * 128],
                                    x_t[:, nch * 512:(nch + 1) * 512],
                                    start=(kc == 0), stop=(kc == KC - 1))
                    for m in range(2):
                        t = proj_sb.tile([128, S], F32R, name=f"{which}hT{m}")
                        nc.scalar.activation(t[:], ps[m][:], AF.Identity,
                                             bias=b_sb[:, m:m + 1], scale=1.0)
                        dst.append(t)

                # vh: psum viewed as 8 x [128t, NS] slices, two [128, S] tensors
                psv = [pps.tile([128, S], F32, name=f"ps_v{i}", tag=f"pp{i}")
                       for i in range(2)]
                for kc in range(KC):
                    x_t = xt.tile([128, S], F32R, name=f"x_v{kc}", tag="xt")
                    nc.gpsimd.dma_start(out=x_t[:], in_=xvT.ap()[kc * 128:(kc + 1) * 128, :])
                    for tb in range(SB):
                        # two [128, 256] regions share each PSUM bank and
                        # start=True clears the WHOLE bank: only the first
                        # region of each bank may set it
                        nc.tensor.matmul(
                            psv[tb // 8][:, (tb % 8) * NS:(tb % 8 + 1) * NS],
                            x_t[:, tb * 128:(tb + 1) * 128],
                            wv_sb[:, kc, :],
                            start=(kc == 0 and tb % 2 == 0), stop=(kc == KC - 1))
                for tb in range(SB):
                    t = proj_sb.tile([128, NS], F32R, name=f"vh{tb}", tag=f"vh{tb}")
                    nc.vector.tensor_add(
                        t[:], psv[tb // 8][:, (tb % 8) * NS:(tb % 8 + 1) * NS], bv_bc[:])
                    vh.append(t)

            # ---------------- attention per head ----------------
            outT = [proj_sb.tile([128, S], F32R, name=f"outT{i}") for i in range(2)]

            for h in range(HPC):
                m, r = h // 2, (h % 2) * 64
                q_h = qhT[m][r:r + 64, :]
                k_h = khT[m][r:r + 64, :]
                rec_mat = smalls.tile([128, SB], F32, name=f"rec{h}", tag="rec")

                # ---- natural side: attn rows + row-sum reciprocals ----
                with tc.tile_pool(name=f"nat_ps{h}", bufs=2, space="PSUM") as nps:
                    for sb in range(SB):
                        ps = nps.tile([128, S], F32, name=f"nat{h}_{sb}", tag="nat")
                        for tch in range(4):
                            nc.tensor.matmul(
                                ps[:, tch * 512:(tch + 1) * 512],
                                q_h[:, sb * 128:(sb + 1) * 128],
                                k_h[:, tch * 512:(tch + 1) * 512],
                                start=True, stop=True)
                        ex = natp.tile([128, S], F32, name=f"exp{h}_{sb}", tag="exp")
                        acc = smalls.tile([128, 1], F32, name=f"acc{h}_{sb}", tag="acc")
                        nc.scalar.activation(ex[:], ps[:], AF.Exp,
                                             scale=float(SCALE), accum_out=acc[:])
                        nc.vector.reciprocal(rec_mat[:, sb:sb + 1], acc[:])
                        at = attp.tile([128, S], F32, name=f"at{h}_{sb}", tag="at")
                        nc.vector.tensor_scalar_mul(at[:], ex[:], rec_mat[:, sb:sb + 1])
                        nc.sync.dma_start(
                            out=attn_d.ap()[h, sb * 128:(sb + 1) * 128, :], in_=at[:])

                # ---- transposed side: exp(scores.T) and AV accumulation ----
                with tc.tile_pool(name=f"t_ps{h}", bufs=2, space="PSUM") as tps, \
                     tc.tile_pool(name=f"av_ps{h}", bufs=1, space="PSUM") as aps:
                    # recip row [1, S] on partition 0
                    rtp = tps.tile([16, 128], F32, name=f"rtp{h}", tag="sT")
                    nc.tensor.transpose(rtp[:], rec_mat[:, 0:16], ident[:])
                    rt_sb = smalls.tile([16, 128], F32, name=f"rt{h}", tag="rt")
                    nc.vector.tensor_copy(rt_sb[:], rtp[:])
                    rrow = smalls.tile([1, S], F32, name=f"rrow{h}", tag="rrow", bufs=1)
                    nc.sync.dma_start(out=rrow[:], in_=rt_sb[0:16, :])

                    av = aps.tile([64, S], F32, name=f"av{h}", tag="av")
                    for sh in range(2):
                        for tb in range(SB):
                            ps = tps.tile([128, 1024], F32, name=f"sT{h}_{sh}_{tb}", tag="sT")
                            for j in range(2):
                                nc.tensor.matmul(
                                    ps[:, j * 512:(j + 1) * 512],
                                    k_h[:, tb * 128:(tb + 1) * 128],
                                    q_h[:, sh * 1024 + j * 512: sh * 1024 + (j + 1) * 512],
                                    start=True, stop=True)
                            et = expTp.tile([128, 1024], F32R, name=f"eT{h}_{sh}_{tb}", tag="eT")
                            nc.scalar.activation(et[:], ps[:], AF.Exp, scale=float(SCALE))
                            for j in range(2):
                                nc.tensor.matmul(
                                    av[:, sh * 1024 + j * 512: sh * 1024 + (j + 1) * 512],
                                    vh[tb][:, h * 64:(h + 1) * 64],
                                    et[:, j * 512:(j + 1) * 512],
                                    start=(tb == 0), stop=(tb == SB - 1))

                    av_raw = avp.tile([64, S], F32, name=f"avr{h}", tag="avr")
                    nc.vector.tensor_copy(av_raw[:], av[:])
                    bc = aps.tile([64, S], F32, name=f"bc{h}", tag="av")
                    for c4 in range(4):
                        nc.tensor.matmul(
                            bc[:, c4 * 512:(c4 + 1) * 512],
                            ones64[0:1, :], rrow[0:1, c4 * 512:(c4 + 1) * 512],
                            start=True, stop=True)
                    if r == 0:
                        nc.vector.tensor_mul(outT[m][0:64, :], av_raw[:], bc[:])
                    else:
                        # DVE lanes cannot shift partitions: normalize in place,
                        # then move rows 0-63 -> 64-127 via DMA (with f32r cast)
                        nc.vector.tensor_mul(av_raw[:], av_raw[:], bc[:])
                        nc.gpsimd.dma_start(out=outT[m][64:128, :],
                                            in_=av_raw[:].bitcast(F32))

            # ---------------- output projection ----------------
            with tc.tile_pool(name="out_ps", bufs=4, space="PSUM") as ops:
                for sb in range(SB):
                    o_sb = outp.tile([128, D], F32, name=f"o{sb}", tag="o")
                    for oc in range(2):
                        ps = ops.tile([128, 512], F32, name=f"po{sb}_{oc}", tag="po")
                        for nch in range(2):
                            nc.tensor.matmul(
                                ps[:],
                                outT[nch][:, sb * 128:(sb + 1) * 128],
                                wo_sb[:, nch, oc * 512:(oc + 1) * 512],
                                start=(nch == 0), stop=(nch == 1))
                        nc.vector.tensor_copy(o_sb[:, oc * 512:(oc + 1) * 512], ps[:])
                    nc.sync.dma_start(out=out_d.ap()[sb * 128:(sb + 1) * 128, :], in_=o_sb[:])

    nc.finalize()
    return nc


def kernel(q, k, v, Wq, bq, Wk, bk, Wv, bv, Wo, bo):
    q, k, v = (np.asarray(x, np.float32) for x in (q, k, v))
    Wq, Wk, Wv, Wo = (np.asarray(x, np.float32) for x in (Wq, Wk, Wv, Wo))
    bq, bk, bv, bo = (np.asarray(x, np.float32) for x in (bq, bk, bv, bo))

    if "nc" not in _CACHE:
        _CACHE["nc"] = _build()
    nc = _CACHE["nc"]

    xqT = [np.ascontiguousarray(q[b].T) for b in range(B)]
    xkT = [np.ascontiguousarray(k[b].T) for b in range(B)]
    xvT = [np.ascontiguousarray(v[b].T) for b in range(B)]

    in_maps = []
    for c in range(NCORES):
        b, g = divmod(c, NCORES // B)
        ns = slice(g * NS, (g + 1) * NS)
        in_maps.append({
            "xqT": xqT[b], "xkT": xkT[b], "xvT": xvT[b],
            "wqT": np.ascontiguousarray(Wq[ns].T),
            "wkT": np.ascontiguousarray(Wk[ns].T),
            "wvT": np.ascontiguousarray(Wv[ns].T),
            "woT": np.ascontiguousarray(Wo[:, ns].T),
            "bq": np.ascontiguousarray(bq[ns]),
            "bk": np.ascontiguousarray(bk[ns]),
            "bv": np.ascontiguousarray(bv[ns]),
        })

    _CACHE["in_maps"] = in_maps
    res = run_bass_kernel_spmd(nc, in_maps, list(range(NCORES))).results
    _CACHE["res"] = res

    attn = np.empty((B, H, S, S), np.float32)
    output = np.zeros((B, S, D), np.float64)
    for c in range(NCORES):
        b, g = divmod(c, NCORES // B)
        attn[b, g * HPC:(g + 1) * HPC] = res[c]["attn_sh"]
        output[b] += res[c]["out_part"].astype(np.float64)
    output += bo.astype(np.float64)
    return output.astype(np.float32), attn


# revision 13
# speedup vs baseline: 1.0298x; 1.0298x over previous
"""MultiHeadAttention Bass/Tile kernel for Trainium2, 8 NeuronCores SPMD.

Problem (hardcoded): B=2, S=2048, D=1024, H=16, d_k=64, fp32.
reference:
    qh = split_heads(q @ Wq.T + bq)  ... etc
    scores = qh @ kh.T / sqrt(d_k)
    attn = softmax(scores)
    out_h = attn @ vh
    output = merge_heads(out_h) @ Wo.T + bo
    returns (output, attn)

Sharding: core c handles batch b=c//4, heads 4*(c%4) .. 4*(c%4)+3
(data parallel over batch x tensor parallel over heads).
W_{q,k,v} split column-wise (256 output dims per core), Wo row-wise;
partial outputs summed on host (cheap: 4 x 8MiB adds per batch).

Per-core pipeline (all matmuls in float32r = hw-rounded fp32, ~1e-4 rel):
  - projections produce qh.T, kh.T ([n,s] layout) and vh ([t,n] layout)
  - per head: scores[s,t] tiles -> exp(scale=1/8) with accum row sums ->
    reciprocal -> normalize -> DMA attn out (natural layout)
  - per head: scores.T[t,s] tiles -> exp -> AV matmul accumulates
    out.T[d,s] (unnormalized); normalized afterwards with a broadcast
    recip row built via PE transpose + ones-outer-product
  - output projection from out.T with Wo.T, partial over this core's heads
"""

import numpy as np

import concourse.bass as bass
import concourse.mybir as mybir
import concourse.tile as tile
from concourse import bacc
from concourse.masks import make_identity
from concourse.bass_utils import run_bass_kernel_spmd

F32 = mybir.dt.float32
F32R = mybir.dt.float32r
AF = mybir.ActivationFunctionType

B, S, D, H, DK = 2, 2048, 1024, 16, 64
NCORES = 8
HPC = 4          # heads per core
NS = HPC * DK    # 256 hidden dims per core
KC = D // 128    # 8 contraction chunks for projections
SB = S // 128    # 16 seq partition blocks
SCALE = 1.0 / np.sqrt(np.float32(DK))

_CACHE = {}


def _build():
    nc = bacc.Bacc()

    xqT = nc.dram_tensor("xqT", [D, S], F32, kind="ExternalInput")
    xkT = nc.dram_tensor("xkT", [D, S], F32, kind="ExternalInput")
    xvT = nc.dram_tensor("xvT", [D, S], F32, kind="ExternalInput")
    wqT = nc.dram_tensor("wqT", [D, NS], F32, kind="ExternalInput")
    wkT = nc.dram_tensor("wkT", [D, NS], F32, kind="ExternalInput")
    wvT = nc.dram_tensor("wvT", [D, NS], F32, kind="ExternalInput")
    woT = nc.dram_tensor("woT", [NS, D], F32, kind="ExternalInput")
    bq_d = nc.dram_tensor("bq", [NS], F32, kind="ExternalInput")
    bk_d = nc.dram_tensor("bk", [NS], F32, kind="ExternalInput")
    bv_d = nc.dram_tensor("bv", [NS], F32, kind="ExternalInput")

    attn_d = nc.dram_tensor("attn_sh", [HPC, S, S], F32, kind="ExternalOutput")
    out_d = nc.dram_tensor("out_part", [S, D], F32, kind="ExternalOutput")

    with tile.TileContext(nc) as tc:
        from contextlib import ExitStack
        with ExitStack() as ctx:
            consts = ctx.enter_context(tc.tile_pool(name="consts", bufs=1))
            wpool = ctx.enter_context(tc.tile_pool(name="wpool", bufs=1))
            xt = ctx.enter_context(tc.tile_pool(name="xt", bufs=2))
            proj_sb = ctx.enter_context(tc.tile_pool(name="proj_sb", bufs=1))
            natp = ctx.enter_context(tc.tile_pool(name="natp", bufs=2))
            attp = ctx.enter_context(tc.tile_pool(name="attp", bufs=2))
            expTp = ctx.enter_context(tc.tile_pool(name="expTp", bufs=3))
            smalls = ctx.enter_context(tc.tile_pool(name="smalls", bufs=2))
            avp = ctx.enter_context(tc.tile_pool(name="avp", bufs=1))
            outp = ctx.enter_context(tc.tile_pool(name="outp", bufs=2))

            # ---------------- constants ----------------
            ident = consts.tile([128, 128], F32)
            make_identity(nc, ident[:])
            ones64 = consts.tile([128, 64], F32)
            nc.vector.memset(ones64[:], 1.0)
            bq_sb = consts.tile([128, 2], F32)
            bk_sb = consts.tile([128, 2], F32)
            nc.sync.dma_start(out=bq_sb[:], in_=bq_d.ap().rearrange("(m p) -> p m", p=128))
            nc.sync.dma_start(out=bk_sb[:], in_=bk_d.ap().rearrange("(m p) -> p m", p=128))
            bv_bc = consts.tile([128, NS], F32)
            bv_src = bass.AP(tensor=bv_d, offset=0, ap=[[0, 128], [1, NS]])
            nc.gpsimd.dma_start(out=bv_bc[:], in_=bv_src)

            # ---------------- weights (cast to f32r) ----------------
            wq_sb = wpool.tile([128, KC, NS], F32R)
            wk_sb = wpool.tile([128, KC, NS], F32R)
            wv_sb = wpool.tile([128, KC, NS], F32R)
            nc.gpsimd.dma_start(out=wq_sb[:], in_=wqT.ap().rearrange("(kc p) n -> p kc n", p=128))
            nc.gpsimd.dma_start(out=wk_sb[:], in_=wkT.ap().rearrange("(kc p) n -> p kc n", p=128))
            nc.gpsimd.dma_start(out=wv_sb[:], in_=wvT.ap().rearrange("(kc p) n -> p kc n", p=128))
            wo_sb = wpool.tile([128, 2, D], F32R)
            nc.gpsimd.dma_start(out=wo_sb[:], in_=woT.ap().rearrange("(nch p) o -> p nch o", p=128))

            # ---------------- projections ----------------
            qhT, khT = [], []   # 2 x [128, S] f32r each ([n, s])
            vh = []             # SB x [128, NS] f32r ([t, n])

            with tc.tile_pool(name="proj_ps", bufs=1, space="PSUM") as pps:
                # qh.T and kh.T : psum[m][n128, s] = sum_kc W*T[kc][:, m].T @ x*T[kc]
                for which, w_sb, x_d, b_sb, dst in (
                    ("q", wq_sb, xqT, bq_sb, qhT),
                    ("k", wk_sb, xkT, bk_sb, khT),
                ):
                    ps = [pps.tile([128, S], F32, name=f"ps_{which}{m}", tag=f"pp{m}")
                          for m in range(2)]
                    for kc in range(KC):
                        x_t = xt.tile([128, S], F32R, name=f"x_{which}{kc}", tag="xt")
                        nc.gpsimd.dma_start(out=x_t[:], in_=x_d.ap()[kc * 128:(kc + 1) * 128, :])
                        for m in range(2):
                            for nch in range(4):
                                nc.tensor.matmul(
                                    ps[m][:, nch * 512:(nch + 1) * 512],
                                    w_sb[:, kc, m * 128:(m + 1) * 128],
                                    x_t[:, nch * 512:(nch + 1) * 512],
                                    start=(kc == 0), stop=(kc == KC - 1))
                    for m in range(2):
                        t = proj_sb.tile([128, S], F32R, name=f"{which}hT{m}")
                        nc.scalar.activation(t[:], ps[m][:], AF.Identity,
                                             bias=b_sb[:, m:m + 1], scale=1.0)
                        dst.append(t)

                # vh: psum viewed as 8 x [128t, NS] slices, two [128, S] tensors
                psv = [pps.tile([128, S], F32, name=f"ps_v{i}", tag=f"pp{i}")
                       for i in range(2)]
                for kc in range(KC):
                    x_t = xt.tile([128, S], F32R, name=f"x_v{kc}", tag="xt")
                    nc.gpsimd.dma_start(out=x_t[:], in_=xvT.ap()[kc * 128:(kc + 1) * 128, :])
                    for tb in range(SB):
                        # two [128, 256] regions share each PSUM bank and
                        # start=True clears the WHOLE bank: only the first
                        # region of each bank may set it
                        nc.tensor.matmul(
                            psv[tb // 8][:, (tb % 8) * NS:(tb % 8 + 1) * NS],
                            x_t[:, tb * 128:(tb + 1) * 128],
                            wv_sb[:, kc, :],
                            start=(kc == 0 and tb % 2 == 0), stop=(kc == KC - 1))
                for tb in range(SB):
                    t = proj_sb.tile([128, NS], F32R, name=f"vh{tb}", tag=f"vh{tb}")
                    nc.vector.tensor_add(
                        t[:], psv[tb // 8][:, (tb % 8) * NS:(tb % 8 + 1) * NS], bv_bc[:])
                    vh.append(t)

            # ---------------- attention per head ----------------
            outT = [proj_sb.tile([128, S], F32R, name=f"outT{i}") for i in range(2)]

            rec_mats = [smalls.tile([128, SB], F32, name=f"rec{i}", tag=f"rec{i}",
                                    bufs=1) for i in range(HPC)]

            # ---- natural side for ALL heads: attn rows + row-sum recips ----
            with tc.tile_pool(name="nat_ps", bufs=2, space="PSUM") as nps:
                for h in range(HPC):
                    m, r = h // 2, (h % 2) * 64
                    q_h = qhT[m][r:r + 64, :]
                    k_h = khT[m][r:r + 64, :]
                    rec_mat = rec_mats[h]
                    for sb in range(SB):
                        ps = nps.tile([128, S], F32, name=f"nat{h}_{sb}", tag="nat")
                        for tch in range(4):
                            nc.tensor.matmul(
                                ps[:, tch * 512:(tch + 1) * 512],
                                q_h[:, sb * 128:(sb + 1) * 128],
                                k_h[:, tch * 512:(tch + 1) * 512],
                                start=True, stop=True)
                        ex = natp.tile([128, S], F32, name=f"exp{h}_{sb}", tag="exp")
                        acc = smalls.tile([128, 1], F32, name=f"acc{h}_{sb}", tag="acc")
                        nc.scalar.activation(ex[:], ps[:], AF.Exp,
                                             scale=float(SCALE), accum_out=acc[:])
                        nc.vector.reciprocal(rec_mat[:, sb:sb + 1], acc[:])
                        at = attp.tile([128, S], F32, name=f"at{h}_{sb}", tag="at")
                        nc.vector.tensor_scalar_mul(at[:], ex[:], rec_mat[:, sb:sb + 1])
                        nc.sync.dma_start(
                            out=attn_d.ap()[h, sb * 128:(sb + 1) * 128, :], in_=at[:])

            # ---- transposed side for ALL heads: exp(scores.T) + AV ----
            with tc.tile_pool(name="t_ps", bufs=2, space="PSUM") as tps, \
                 tc.tile_pool(name="av_ps", bufs=1, space="PSUM") as aps:
                for h in range(HPC):
                    m, r = h // 2, (h % 2) * 64
                    q_h = qhT[m][r:r + 64, :]
                    k_h = khT[m][r:r + 64, :]
                    rec_mat = rec_mats[h]
                    # recip row [1, S] on partition 0
                    rtp = tps.tile([16, 128], F32, name=f"rtp{h}", tag="sT")
                    nc.tensor.transpose(rtp[:], rec_mat[:, 0:16], ident[:])
                    rt_sb = smalls.tile([16, 128], F32, name=f"rt{h}", tag="rt")
                    nc.vector.tensor_copy(rt_sb[:], rtp[:])
                    rrow = smalls.tile([1, S], F32, name=f"rrow{h}", tag="rrow", bufs=1)
                    nc.sync.dma_start(out=rrow[:], in_=rt_sb[0:16, :])

                    av = aps.tile([64, S], F32, name=f"av{h}", tag="av")
                    for sh in range(2):
                        for tb in range(SB):
                            ps = tps.tile([128, 1024], F32, name=f"sT{h}_{sh}_{tb}", tag="sT")
                            for j in range(2):
                                nc.tensor.matmul(
                                    ps[:, j * 512:(j + 1) * 512],
                                    k_h[:, tb * 128:(tb + 1) * 128],
                                    q_h[:, sh * 1024 + j * 512: sh * 1024 + (j + 1) * 512],
                                    start=True, stop=True)
                            et = expTp.tile([128, 1024], F32R, name=f"eT{h}_{sh}_{tb}", tag="eT")
                            nc.scalar.activation(et[:], ps[:], AF.Exp, scale=float(SCALE))
                            for j in range(2):
                                nc.tensor.matmul(
                                    av[:, sh * 1024 + j * 512: sh * 1024 + (j + 1) * 512],
                                    vh[tb][:, h * 64:(h + 1) * 64],
                                    et[:, j * 512:(j + 1) * 512],
                                    start=(tb == 0), stop=(tb == SB - 1))

                    av_raw = avp.tile([64, S], F32, name=f"avr{h}", tag="avr")
                    nc.vector.tensor_copy(av_raw[:], av[:])
                    bc = aps.tile([64, S], F32, name=f"bc{h}", tag="av")
                    for c4 in range(4):
                        nc.tensor.matmul(
                            bc[:, c4 * 512:(c4 + 1) * 512],
                            ones64[0:1, :], rrow[0:1, c4 * 512:(c4 + 1) * 512],
                            start=True, stop=True)
                    if r == 0:
                        nc.vector.tensor_mul(outT[m][0:64, :], av_raw[:], bc[:])
                    else:
                        # DVE lanes cannot shift partitions: normalize in place,
                        # then move rows 0-63 -> 64-127 via DMA (with f32r cast)
                        nc.vector.tensor_mul(av_raw[:], av_raw[:], bc[:])
                        nc.gpsimd.dma_start(out=outT[m][64:128, :],
                                            in_=av_raw[:].bitcast(F32))

            # ---------------- output projection ----------------
            with tc.tile_pool(name="out_ps", bufs=4, space="PSUM") as ops:
                for sb in range(SB):
                    o_sb = outp.tile([128, D], F32, name=f"o{sb}", tag="o")
                    for oc in range(2):
                        ps = ops.tile([128, 512], F32, name=f"po{sb}_{oc}", tag="po")
                        for nch in range(2):
                            nc.tensor.matmul(
                                ps[:],
                                outT[nch][:, sb * 128:(sb + 1) * 128],
                                wo_sb[:, nch, oc * 512:(oc + 1) * 512],
                                start=(nch == 0), stop=(nch == 1))
                        nc.vector.tensor_copy(o_sb[:, oc * 512:(oc + 1) * 512], ps[:])
                    nc.sync.dma_start(out=out_d.ap()[sb * 128:(sb + 1) * 128, :], in_=o_sb[:])

    nc.finalize()
    return nc


def kernel(q, k, v, Wq, bq, Wk, bk, Wv, bv, Wo, bo):
    q, k, v = (np.asarray(x, np.float32) for x in (q, k, v))
    Wq, Wk, Wv, Wo = (np.asarray(x, np.float32) for x in (Wq, Wk, Wv, Wo))
    bq, bk, bv, bo = (np.asarray(x, np.float32) for x in (bq, bk, bv, bo))

    if "nc" not in _CACHE:
        _CACHE["nc"] = _build()
    nc = _CACHE["nc"]

    xqT = [np.ascontiguousarray(q[b].T) for b in range(B)]
    xkT = [np.ascontiguousarray(k[b].T) for b in range(B)]
    xvT = [np.ascontiguousarray(v[b].T) for b in range(B)]

    in_maps = []
    for c in range(NCORES):
        b, g = divmod(c, NCORES // B)
        ns = slice(g * NS, (g + 1) * NS)
        in_maps.append({
            "xqT": xqT[b], "xkT": xkT[b], "xvT": xvT[b],
            "wqT": np.ascontiguousarray(Wq[ns].T),
            "wkT": np.ascontiguousarray(Wk[ns].T),
            "wvT": np.ascontiguousarray(Wv[ns].T),
            "woT": np.ascontiguousarray(Wo[:, ns].T),
            "bq": np.ascontiguousarray(bq[ns]),
            "bk": np.ascontiguousarray(bk[ns]),
            "bv": np.ascontiguousarray(bv[ns]),
        })

    _CACHE["in_maps"] = in_maps
    res = run_bass_kernel_spmd(nc, in_maps, list(range(NCORES))).results
    _CACHE["res"] = res

    attn = np.empty((B, H, S, S), np.float32)
    output = np.zeros((B, S, D), np.float64)
    for c in range(NCORES):
        b, g = divmod(c, NCORES // B)
        attn[b, g * HPC:(g + 1) * HPC] = res[c]["attn_sh"]
        output[b] += res[c]["out_part"].astype(np.float64)
    output += bo.astype(np.float64)
    return output.astype(np.float32), attn


# revision 14
# speedup vs baseline: 1.1372x; 1.1043x over previous
"""MultiHeadAttention Bass/Tile kernel for Trainium2, 8 NeuronCores SPMD.

Problem (hardcoded): B=2, S=2048, D=1024, H=16, d_k=64, fp32.
reference:
    qh = split_heads(q @ Wq.T + bq)  ... etc
    scores = qh @ kh.T / sqrt(d_k)
    attn = softmax(scores)
    out_h = attn @ vh
    output = merge_heads(out_h) @ Wo.T + bo
    returns (output, attn)

Sharding: core c handles batch b=c//4, heads 4*(c%4) .. 4*(c%4)+3
(data parallel over batch x tensor parallel over heads).
W_{q,k,v} split column-wise (256 output dims per core), Wo row-wise;
partial outputs summed on host (cheap: 4 x 8MiB adds per batch).

Per-core pipeline (all matmuls in float32r = hw-rounded fp32, ~1e-4 rel):
  - projections produce qh.T, kh.T ([n,s] layout) and vh ([t,n] layout)
  - per head: scores[s,t] tiles -> exp(scale=1/8) with accum row sums ->
    reciprocal -> normalize -> DMA attn out (natural layout)
  - per head: scores.T[t,s] tiles -> exp -> AV matmul accumulates
    out.T[d,s] (unnormalized); normalized afterwards with a broadcast
    recip row built via PE transpose + ones-outer-product
  - output projection from out.T with Wo.T, partial over this core's heads
"""

import numpy as np

import concourse.bass as bass
import concourse.mybir as mybir
import concourse.tile as tile
from concourse import bacc
from concourse.masks import make_identity
from concourse.bass_utils import run_bass_kernel_spmd

F32 = mybir.dt.float32
F32R = mybir.dt.float32r
AF = mybir.ActivationFunctionType

B, S, D, H, DK = 2, 2048, 1024, 16, 64
NCORES = 8
HPC = 4          # heads per core
NS = HPC * DK    # 256 hidden dims per core
KC = D // 128    # 8 contraction chunks for projections
SB = S // 128    # 16 seq partition blocks
SCALE = 1.0 / np.sqrt(np.float32(DK))

_CACHE = {}


def _build():
    nc = bacc.Bacc()

    xqT = nc.dram_tensor("xqT", [D, S], F32, kind="ExternalInput")
    xkT = nc.dram_tensor("xkT", [D, S], F32, kind="ExternalInput")
    xvT = nc.dram_tensor("xvT", [D, S], F32, kind="ExternalInput")
    wqT = nc.dram_tensor("wqT", [D, NS], F32, kind="ExternalInput")
    wkT = nc.dram_tensor("wkT", [D, NS], F32, kind="ExternalInput")
    wvT = nc.dram_tensor("wvT", [D, NS], F32, kind="ExternalInput")
    woT = nc.dram_tensor("woT", [NS, D], F32, kind="ExternalInput")
    bq_d = nc.dram_tensor("bq", [NS], F32, kind="ExternalInput")
    bk_d = nc.dram_tensor("bk", [NS], F32, kind="ExternalInput")
    bv_d = nc.dram_tensor("bv", [NS], F32, kind="ExternalInput")

    attn_d = nc.dram_tensor("attn_sh", [HPC, S, S], F32, kind="ExternalOutput")
    out_d = nc.dram_tensor("out_part", [S, D], F32, kind="ExternalOutput")

    with tile.TileContext(nc) as tc:
        from contextlib import ExitStack
        with ExitStack() as ctx:
            consts = ctx.enter_context(tc.tile_pool(name="consts", bufs=1))
            wpool = ctx.enter_context(tc.tile_pool(name="wpool", bufs=1))
            xt = ctx.enter_context(tc.tile_pool(name="xt", bufs=3))
            proj_sb = ctx.enter_context(tc.tile_pool(name="proj_sb", bufs=1))
            natp = ctx.enter_context(tc.tile_pool(name="natp", bufs=2))
            attp = ctx.enter_context(tc.tile_pool(name="attp", bufs=2))
            expTp = ctx.enter_context(tc.tile_pool(name="expTp", bufs=3))
            smalls = ctx.enter_context(tc.tile_pool(name="smalls", bufs=2))
            avp = ctx.enter_context(tc.tile_pool(name="avp", bufs=2))
            outp = ctx.enter_context(tc.tile_pool(name="outp", bufs=2))

            # ---------------- constants ----------------
            ident = consts.tile([128, 128], F32)
            make_identity(nc, ident[:])
            ones64 = consts.tile([128, 64], F32)
            nc.vector.memset(ones64[:], 1.0)
            bq_sb = consts.tile([128, 2], F32)
            bk_sb = consts.tile([128, 2], F32)
            nc.sync.dma_start(out=bq_sb[:], in_=bq_d.ap().rearrange("(m p) -> p m", p=128))
            nc.sync.dma_start(out=bk_sb[:], in_=bk_d.ap().rearrange("(m p) -> p m", p=128))
            bv_bc = consts.tile([128, NS], F32)
            bv_src = bass.AP(tensor=bv_d, offset=0, ap=[[0, 128], [1, NS]])
            nc.gpsimd.dma_start(out=bv_bc[:], in_=bv_src)

            # ---------------- weights (cast to f32r) ----------------
            wq_sb = wpool.tile([128, KC, NS], F32R)
            wk_sb = wpool.tile([128, KC, NS], F32R)
            wv_sb = wpool.tile([128, KC, NS], F32R)
            nc.gpsimd.dma_start(out=wq_sb[:], in_=wqT.ap().rearrange("(kc p) n -> p kc n", p=128))
            nc.gpsimd.dma_start(out=wk_sb[:], in_=wkT.ap().rearrange("(kc p) n -> p kc n", p=128))
            nc.gpsimd.dma_start(out=wv_sb[:], in_=wvT.ap().rearrange("(kc p) n -> p kc n", p=128))
            wo_sb = wpool.tile([128, 2, D], F32R)
            nc.gpsimd.dma_start(out=wo_sb[:], in_=woT.ap().rearrange("(nch p) o -> p nch o", p=128))

            # ---------------- projections ----------------
            qhT, khT = [], []   # 2 x [128, S] f32r each ([n, s])
            vh = []             # SB x [128, NS] f32r ([t, n])

            with tc.tile_pool(name="proj_ps", bufs=1, space="PSUM") as pps:
                # qh.T and kh.T : psum[m][n128, s] = sum_kc W*T[kc][:, m].T @ x*T[kc]
                for which, w_sb, x_d, b_sb, dst in (
                    ("q", wq_sb, xqT, bq_sb, qhT),
                    ("k", wk_sb, xkT, bk_sb, khT),
                ):
                    ps = [pps.tile([128, S], F32, name=f"ps_{which}{m}", tag=f"pp{m}")
                          for m in range(2)]
                    for kc in range(KC):
                        x_t = xt.tile([128, S], F32R, name=f"x_{which}{kc}", tag="xt")
                        nc.gpsimd.dma_start(out=x_t[:], in_=x_d.ap()[kc * 128:(kc + 1) * 128, :])
                        for m in range(2):
                            for nch in range(4):
                                nc.tensor.matmul(
                                    ps[m][:, nch * 512:(nch + 1) * 512],
                                    w_sb[:, kc, m * 128:(m + 1) * 128],
                                    x_t[:, nch * 512:(nch + 1) * 512],
                                    start=(kc == 0), stop=(kc == KC - 1))
                    for m in range(2):
                        t = proj_sb.tile([128, S], F32R, name=f"{which}hT{m}")
                        nc.scalar.activation(t[:], ps[m][:], AF.Identity,
                                             bias=b_sb[:, m:m + 1], scale=1.0)
                        dst.append(t)

                # vh: psum viewed as 8 x [128t, NS] slices, two [128, S] tensors
                psv = [pps.tile([128, S], F32, name=f"ps_v{i}", tag=f"pp{i}")
                       for i in range(2)]
                for kc in range(KC):
                    x_t = xt.tile([128, S], F32R, name=f"x_v{kc}", tag="xt")
                    nc.gpsimd.dma_start(out=x_t[:], in_=xvT.ap()[kc * 128:(kc + 1) * 128, :])
                    for tb in range(SB):
                        # two [128, 256] regions share each PSUM bank and
                        # start=True clears the WHOLE bank: only the first
                        # region of each bank may set it
                        nc.tensor.matmul(
                            psv[tb // 8][:, (tb % 8) * NS:(tb % 8 + 1) * NS],
                            x_t[:, tb * 128:(tb + 1) * 128],
                            wv_sb[:, kc, :],
                            start=(kc == 0 and tb % 2 == 0), stop=(kc == KC - 1))
                for tb in range(SB):
                    t = proj_sb.tile([128, NS], F32R, name=f"vh{tb}", tag=f"vh{tb}")
                    nc.vector.tensor_add(
                        t[:], psv[tb // 8][:, (tb % 8) * NS:(tb % 8 + 1) * NS], bv_bc[:])
                    vh.append(t)

            # ---------------- attention per head ----------------
            outT = [proj_sb.tile([128, S], F32R, name=f"outT{i}") for i in range(2)]

            rec_mats = [smalls.tile([128, SB], F32, name=f"rec{i}", tag=f"rec{i}",
                                    bufs=1) for i in range(HPC)]

            # ---- natural side for ALL heads: attn rows + row-sum recips ----
            with tc.tile_pool(name="nat_ps", bufs=2, space="PSUM") as nps:
                for h in range(HPC):
                    m, r = h // 2, (h % 2) * 64
                    q_h = qhT[m][r:r + 64, :]
                    k_h = khT[m][r:r + 64, :]
                    rec_mat = rec_mats[h]
                    for sb in range(SB):
                        ps = nps.tile([128, S], F32, name=f"nat{h}_{sb}", tag="nat")
                        for tch in range(4):
                            nc.tensor.matmul(
                                ps[:, tch * 512:(tch + 1) * 512],
                                q_h[:, sb * 128:(sb + 1) * 128],
                                k_h[:, tch * 512:(tch + 1) * 512],
                                start=True, stop=True)
                        ex = natp.tile([128, S], F32, name=f"exp{h}_{sb}", tag="exp")
                        acc = smalls.tile([128, 1], F32, name=f"acc{h}_{sb}", tag="acc")
                        nc.scalar.activation(ex[:], ps[:], AF.Exp,
                                             scale=float(SCALE), accum_out=acc[:])
                        nc.vector.reciprocal(rec_mat[:, sb:sb + 1], acc[:])
                        at = attp.tile([128, S], F32, name=f"at{h}_{sb}", tag="at")
                        nc.vector.tensor_scalar_mul(at[:], ex[:], rec_mat[:, sb:sb + 1])
                        nc.sync.dma_start(
                            out=attn_d.ap()[h, sb * 128:(sb + 1) * 128, :], in_=at[:])

            # ---- transposed side for ALL heads: exp(scores.T) + AV ----
            with tc.tile_pool(name="t_ps", bufs=2, space="PSUM") as tps, \
                 tc.tile_pool(name="av_ps", bufs=1, space="PSUM") as aps:
                for h in range(HPC):
                    m, r = h // 2, (h % 2) * 64
                    q_h = qhT[m][r:r + 64, :]
                    k_h = khT[m][r:r + 64, :]
                    rec_mat = rec_mats[h]
                    # recip row [1, S] on partition 0
                    rtp = tps.tile([16, 128], F32, name=f"rtp{h}", tag="sT")
                    nc.tensor.transpose(rtp[:], rec_mat[:, 0:16], ident[:])
                    rt_sb = smalls.tile([16, 128], F32, name=f"rt{h}", tag="rt")
                    nc.vector.tensor_copy(rt_sb[:], rtp[:])
                    rrow = smalls.tile([1, S], F32, name=f"rrow{h}", tag="rrow", bufs=1)
                    nc.sync.dma_start(out=rrow[:], in_=rt_sb[0:16, :])

                    av = aps.tile([64, S], F32, name=f"av{h}", tag="av")
                    for sh in range(2):
                        for tb in range(SB):
                            ps = tps.tile([128, 1024], F32, name=f"sT{h}_{sh}_{tb}", tag="sT")
                            for j in range(2):
                                nc.tensor.matmul(
                                    ps[:, j * 512:(j + 1) * 512],
                                    k_h[:, tb * 128:(tb + 1) * 128],
                                    q_h[:, sh * 1024 + j * 512: sh * 1024 + (j + 1) * 512],
                                    start=True, stop=True)
                            et = expTp.tile([128, 1024], F32R, name=f"eT{h}_{sh}_{tb}", tag="eT")
                            nc.scalar.activation(et[:], ps[:], AF.Exp, scale=float(SCALE))
                            for j in range(2):
                                nc.tensor.matmul(
                                    av[:, sh * 1024 + j * 512: sh * 1024 + (j + 1) * 512],
                                    vh[tb][:, h * 64:(h + 1) * 64],
                                    et[:, j * 512:(j + 1) * 512],
                                    start=(tb == 0), stop=(tb == SB - 1))

                    av_raw = avp.tile([64, S], F32, name=f"avr{h}", tag="avr")
                    nc.vector.tensor_copy(av_raw[:], av[:])
                    bc = aps.tile([64, S], F32, name=f"bc{h}", tag="av")
                    for c4 in range(4):
                        nc.tensor.matmul(
                            bc[:, c4 * 512:(c4 + 1) * 512],
                            ones64[0:1, :], rrow[0:1, c4 * 512:(c4 + 1) * 512],
                            start=True, stop=True)
                    if r == 0:
                        nc.vector.tensor_mul(outT[m][0:64, :], av_raw[:], bc[:])
                    else:
                        # DVE lanes cannot shift partitions: normalize in place,
                        # then move rows 0-63 -> 64-127 via DMA (with f32r cast)
                        nc.vector.tensor_mul(av_raw[:], av_raw[:], bc[:])
                        nc.gpsimd.dma_start(out=outT[m][64:128, :],
                                            in_=av_raw[:].bitcast(F32))

            # ---------------- output projection ----------------
            with tc.tile_pool(name="out_ps", bufs=4, space="PSUM") as ops:
                for sb in range(SB):
                    o_sb = outp.tile([128, D], F32, name=f"o{sb}", tag="o")
                    for oc in range(2):
                        ps = ops.tile([128, 512], F32, name=f"po{sb}_{oc}", tag="po")
                        for nch in range(2):
                            nc.tensor.matmul(
                                ps[:],
                                outT[nch][:, sb * 128:(sb + 1) * 128],
                                wo_sb[:, nch, oc * 512:(oc + 1) * 512],
                                start=(nch == 0), stop=(nch == 1))
                        nc.vector.tensor_copy(o_sb[:, oc * 512:(oc + 1) * 512], ps[:])
                    nc.sync.dma_start(out=out_d.ap()[sb * 128:(sb + 1) * 128, :], in_=o_sb[:])

    nc.finalize()
    return nc


def kernel(q, k, v, Wq, bq, Wk, bk, Wv, bv, Wo, bo):
    q, k, v = (np.asarray(x, np.float32) for x in (q, k, v))
    Wq, Wk, Wv, Wo = (np.asarray(x, np.float32) for x in (Wq, Wk, Wv, Wo))
    bq, bk, bv, bo = (np.asarray(x, np.float32) for x in (bq, bk, bv, bo))

    if "nc" not in _CACHE:
        _CACHE["nc"] = _build()
    nc = _CACHE["nc"]

    xqT = [np.ascontiguousarray(q[b].T) for b in range(B)]
    xkT = [np.ascontiguousarray(k[b].T) for b in range(B)]
    xvT = [np.ascontiguousarray(v[b].T) for b in range(B)]

    in_maps = []
    for c in range(NCORES):
        b, g = divmod(c, NCORES // B)
        ns = slice(g * NS, (g + 1) * NS)
        in_maps.append({
            "xqT": xqT[b], "xkT": xkT[b], "xvT": xvT[b],
            "wqT": np.ascontiguousarray(Wq[ns].T),
            "wkT": np.ascontiguousarray(Wk[ns].T),
            "wvT": np.ascontiguousarray(Wv[ns].T),
            "woT": np.ascontiguousarray(Wo[:, ns].T),
            "bq": np.ascontiguousarray(bq[ns]),
            "bk": np.ascontiguousarray(bk[ns]),
            "bv": np.ascontiguousarray(bv[ns]),
        })

    _CACHE["in_maps"] = in_maps
    res = run_bass_kernel_spmd(nc, in_maps, list(range(NCORES))).results
    _CACHE["res"] = res

    attn = np.empty((B, H, S, S), np.float32)
    output = np.zeros((B, S, D), np.float64)
    for c in range(NCORES):
        b, g = divmod(c, NCORES // B)
        attn[b, g * HPC:(g + 1) * HPC] = res[c]["attn_sh"]
        output[b] += res[c]["out_part"].astype(np.float64)
    output += bo.astype(np.float64)
    return output.astype(np.float32), attn
